# revision 10
# baseline (speedup 1.0000x reference)
"""Trainium2 Bass kernel for CausalGNNRecommender.

Full inputs in, full outputs out. Internally: shard the N=16384 node dim
across 8 NeuronCores (2048 rows each, degree-sorted within each core for
block-ELL edge padding), run a fused attention + 2x SAGE-conv kernel per
core with AllGather collectives between layers, then unshard on host.
"""

import sys

sys.path.insert(0, "/opt/trn_rl_repo")

import numpy as np

import concourse.bacc as bacc
import concourse.tile as tile
from concourse import mybir
from concourse.bass_utils import run_bass_kernel_spmd
from concourse.masks import make_identity

# Problem shapes (hardcoded per harness contract).
U, I, H, E = 4096, 12288, 64, 524288
N = U + I              # 16384
CORES = 8
NL = N // CORES        # 2048 rows per core
NB = NL // 128         # 16 node blocks per core
KC = N // 128          # 128 key chunks
G = NL // 512          # 4 groups of 512 rows
PAD_ROW = N            # gather-table row of zeros used for ELL pad slots

f32 = mybir.dt.float32
i16 = mybir.dt.int16

# Results of the last device run (exec time etc.) for external inspection.
LAST_RESULTS = None
_NC_CACHE = {}


def _build_nc(C_total, Lj, offs, chunks, segs):
    """Build the single-core SPMD Bass program (identical on all 8 cores)."""
    nc = bacc.Bacc("TRN2", target_bir_lowering=False, debug=False)
    S = 128 * C_total

    t_x0aug = nc.dram_tensor("x0aug", [N, H + 1], f32, kind="ExternalInput")
    t_xT0 = nc.dram_tensor("xT0", [H, NL], f32, kind="ExternalInput")
    t_adj = nc.dram_tensor("adj", [H, N], f32, kind="ExternalInput")
    t_wpack = nc.dram_tensor("wpack", [H, 323], f32, kind="ExternalInput")
    t_idxw = nc.dram_tensor("idxw", [128, S // 16], i16, kind="ExternalInput")
    t_idxs = nc.dram_tensor("idxs", [128, C_total], i16, kind="ExternalInput")
    t_out = nc.dram_tensor("out", [NL, H], f32, kind="ExternalOutput")

    d_sh = [nc.dram_tensor(f"sh{l}", [NL, H], f32) for l in range(2)]
    d_tb = [
        nc.dram_tensor(f"tb{l}", [N + 128, H], f32, addr_space="Shared")
        for l in range(2)
    ]

    WC_ATTN = slice(0, 64)
    WC_LW = [slice(64, 128), slice(192, 256)]
    WC_RW = [slice(128, 192), slice(256, 320)]
    WC_AB = slice(320, 321)
    WC_LB = [slice(321, 322), slice(322, 323)]

    with tile.TileContext(nc) as tc:
        with tc.tile_pool(name="consts", bufs=1) as consts:
            ident = consts.tile([128, 128], f32)
            make_identity(nc, ident[:])
            wpack = consts.tile([H, 323], f32)
            nc.sync.dma_start(wpack[:], t_wpack[:])
            idxw = consts.tile([128, S // 16], i16)
            nc.sync.dma_start(idxw[:], t_idxw[:])
            idxs = consts.tile([128, C_total], i16)
            nc.sync.dma_start(idxs[:], t_idxs[:])
            xT0 = consts.tile([H, NL], f32)
            nc.sync.dma_start(xT0[:], t_xT0[:])
            drecip = consts.tile([128, NB], f32)
            x_a = consts.tile([H, NL], f32)  # x1T (attention out, col layout)
            x_b = consts.tile([H, NL], f32)  # x2T (layer-0 out, col layout)

            # Zero pad rows [N, N+128) of both gather tables.
            ztile = consts.tile([128, H], f32)
            nc.gpsimd.memset(ztile[:], 0.0)
            for l in range(2):
                nc.sync.dma_start(
                    d_tb[l].ap().rearrange("(c p) h -> p c h", p=128)[:, N // 128, :],
                    ztile[:],
                )

            # ---------------- attention ----------------
            with (
                tc.tile_pool(name="attn_sb", bufs=1) as asb,
                tc.tile_pool(name="exp_sb", bufs=8) as esb,
                tc.tile_pool(name="ps_out", bufs=1, space="PSUM") as pso,
            ):
                x0aug = asb.tile([128, KC, H + 1], f32)
                nc.sync.dma_start(
                    x0aug[:], t_x0aug.ap().rearrange("(k p) h -> p k h", p=128)
                )
                adj = asb.tile([H, N], f32)
                nc.sync.dma_start(adj[:], t_adj[:])

                # xWT = attn_w @ xT0 + b   [64, NL]
                xWT = asb.tile([H, NL], f32)
                with tc.tile_pool(name="ps_w", bufs=2, space="PSUM") as psw:
                    for g in range(G):
                        gs = slice(g * 512, (g + 1) * 512)
                        ps = psw.tile([H, 512], f32, tag="xw")
                        nc.tensor.matmul(ps[:], wpack[:, WC_ATTN], xT0[:, gs])
                        nc.scalar.activation(
                            xWT[:, gs],
                            ps[:],
                            mybir.ActivationFunctionType.Identity,
                            bias=wpack[:, WC_AB],
                        )

                out_ps = [
                    pso.tile([H + 1, 512], f32, tag=f"o{g}", name=f"out_ps{g}")
                    for g in range(G)
                ]

                with tc.tile_pool(name="ps_sc", bufs=3, space="PSUM") as pss:
                    for k in range(KC):
                        exs = []
                        for g in range(G):
                            gs = slice(g * 512, (g + 1) * 512)
                            sc = pss.tile([128, 512], f32, tag="sc")
                            nc.tensor.matmul(
                                sc[:], adj[:, k * 128 : (k + 1) * 128], xWT[:, gs]
                            )
                            ex = esb.tile([128, 512], f32, tag="ex")
                            nc.scalar.activation(
                                ex[:], sc[:], mybir.ActivationFunctionType.Exp
                            )
                            exs.append(ex)
                        for g in range(G):
                            nc.tensor.matmul(
                                out_ps[g][:],
                                x0aug[:, k, :],
                                exs[g][:],
                                start=(k == 0),
                                stop=(k == KC - 1),
                            )

                # softmax divide + build x1 rows and x1T
                x1rows = asb.tile([128, NB, H], f32)
                with tc.tile_pool(name="ps_tr", bufs=2, space="PSUM") as pst:
                    for g in range(G):
                        ot = esb.tile([H + 1, 512], f32, tag="ot")
                        nc.scalar.copy(ot[:], out_ps[g][:])
                        for i in range(4):
                            j = g * 4 + i
                            pr = pst.tile([128, H + 1], f32, tag="pr")
                            nc.tensor.transpose(
                                pr[:],
                                ot[:, i * 128 : (i + 1) * 128],
                                ident[0 : H + 1, 0 : H + 1],
                            )
                            r = esb.tile([128, 1], f32, tag="r")
                            nc.vector.reciprocal(r[:], pr[:, H : H + 1])
                            nc.vector.tensor_scalar_mul(
                                x1rows[:, j, :], pr[:, 0:H], r[:]
                            )
                            pt = pst.tile([H, 128], f32, tag="pt")
                            nc.tensor.transpose(pt[:], x1rows[:, j, :], ident[:])
                            nc.vector.tensor_copy(
                                x_a[:, j * 128 : (j + 1) * 128], pt[:]
                            )
                nc.sync.dma_start(
                    d_sh[0].ap().rearrange("(c p) h -> p c h", p=128), x1rows[:]
                )

            # ---------------- SAGE layers ----------------
            with (
                tc.tile_pool(name="sage_sb", bufs=1) as ssb,
                tc.tile_pool(name="gat_sb", bufs=3) as gsb,
                tc.tile_pool(name="ps_sage", bufs=2, space="PSUM") as ps2p,
                tc.tile_pool(name="ps_str", bufs=2, space="PSUM") as ps2t,
            ):
                for layer in range(2):
                    x_in = x_a if layer == 0 else x_b
                    x_out = x_b if layer == 0 else x_a

                    nc.gpsimd.collective_compute(
                        "AllGather",
                        mybir.AluOpType.bypass,
                        replica_groups=[list(range(CORES))],
                        ins=[d_sh[layer][:]],
                        outs=[d_tb[layer][0:N, :]],
                    )

                    # chunked gather (<=8192 idxs per dma_gather: SWDGE ring cap)
                    # with per-block segment reduces; a block may span two
                    # consecutive chunks (partial reduce + add).
                    aggr = ssb.tile([128, NB, H], f32, tag=f"aggr{layer}")
                    gt_tiles = {}
                    for ci, (c0, c1) in enumerate(chunks):
                        clen = c1 - c0
                        ns = 128 * clen
                        gt = gsb.tile(
                            [128, clen, H], f32, tag="gt", name=f"gt{layer}_{ci}"
                        )
                        gt_tiles[ci] = (gt, c0)
                        nc.gpsimd.dma_gather(
                            gt[:],
                            d_tb[layer][:],
                            idxw[:, 8 * c0 : 8 * c1],
                            ns,
                            ns,
                            H,
                            single_packet=False,
                        )
                        for j in range(NB):
                            if not segs[j] or segs[j][-1][0] != ci:
                                continue
                            for si, (sci, rel, ln) in enumerate(segs[j]):
                                sgt, _ = gt_tiles[sci]
                                red_in = sgt[:, rel : rel + ln, :].rearrange(
                                    "p l h -> p h l"
                                )
                                if si == 0:
                                    nc.vector.tensor_reduce(
                                        aggr[:, j, :],
                                        red_in,
                                        axis=mybir.AxisListType.X,
                                        op=mybir.AluOpType.add,
                                    )
                                else:
                                    tmp = gsb.tile(
                                        [128, H], f32, tag="rtmp",
                                        name=f"rtmp{layer}_{j}",
                                    )
                                    nc.vector.tensor_reduce(
                                        tmp[:],
                                        red_in,
                                        axis=mybir.AxisListType.X,
                                        op=mybir.AluOpType.add,
                                    )
                                    nc.vector.tensor_tensor(
                                        aggr[:, j, :],
                                        aggr[:, j, :],
                                        tmp[:],
                                        op=mybir.AluOpType.add,
                                    )

                    if layer == 0:
                        # deg[n] = number of real (non-pad) slots; pad slots
                        # hold idx == N, real slots hold idx < N.
                        indf = ssb.tile([128, C_total], f32)
                        nc.vector.tensor_copy(indf[:], idxs[:])
                        ind2 = ssb.tile([128, C_total], f32)
                        nc.vector.tensor_scalar(
                            ind2[:],
                            indf[:],
                            float(N) - 0.5,
                            None,
                            op0=mybir.AluOpType.is_lt,
                        )
                        degt = ssb.tile([128, NB], f32)
                        for j in range(NB):
                            nc.vector.tensor_reduce(
                                degt[:, j : j + 1],
                                ind2[:, offs[j] : offs[j + 1]],
                                axis=mybir.AxisListType.X,
                                op=mybir.AluOpType.add,
                            )
                        nc.vector.tensor_scalar_max(degt[:], degt[:], 1.0)
                        nc.vector.reciprocal(drecip[:], degt[:])

                    aggrT = ssb.tile([H, NL], f32, tag=f"aggrT{layer}")
                    for j in range(NB):
                        nc.vector.tensor_scalar_mul(
                            aggr[:, j, :], aggr[:, j, :], drecip[:, j : j + 1]
                        )
                        pt = ps2t.tile([H, 128], f32, tag="pt2")
                        nc.tensor.transpose(pt[:], aggr[:, j, :], ident[:])
                        nc.vector.tensor_copy(
                            aggrT[:, j * 128 : (j + 1) * 128], pt[:]
                        )

                    for g in range(G):
                        gs = slice(g * 512, (g + 1) * 512)
                        ps2 = ps2p.tile([H, 512], f32, tag="sage")
                        nc.tensor.matmul(
                            ps2[:], wpack[:, WC_LW[layer]], aggrT[:, gs],
                            start=True, stop=False,
                        )
                        nc.tensor.matmul(
                            ps2[:], wpack[:, WC_RW[layer]], x_in[:, gs],
                            start=False, stop=True,
                        )
                        nc.scalar.activation(
                            x_out[:, gs],
                            ps2[:],
                            mybir.ActivationFunctionType.Relu,
                            bias=wpack[:, WC_LB[layer]],
                        )

                    xrows = ssb.tile([128, NB, H], f32, tag=f"xrows{layer}")
                    for j in range(NB):
                        pr2 = ps2t.tile([128, H], f32, tag="pr2")
                        nc.tensor.transpose(
                            pr2[:],
                            x_out[:, j * 128 : (j + 1) * 128],
                            ident[0:H, 0:H],
                        )
                        nc.vector.tensor_copy(xrows[:, j, :], pr2[:])
                    dst = d_sh[1] if layer == 0 else t_out
                    nc.sync.dma_start(
                        dst.ap().rearrange("(c p) h -> p c h", p=128), xrows[:]
                    )

    nc.finalize()
    return nc


def _preprocess(edge_index):
    """Edge/graph preprocessing: per-core degree-sorted node permutation and
    block-ELL slot encoding of the edge list (partitioned by destination)."""
    src = np.asarray(edge_index[0], dtype=np.int64)
    tgt = np.asarray(edge_index[1], dtype=np.int64)
    deg = np.bincount(tgt, minlength=N)

    perm = np.empty(N, dtype=np.int64)
    for c in range(CORES):
        seg = np.arange(c * NL, (c + 1) * NL)
        order = np.argsort(-deg[seg], kind="stable")
        perm[c * NL : (c + 1) * NL] = seg[order]
    pos_of = np.empty(N, dtype=np.int64)
    pos_of[perm] = np.arange(N)

    deg_pos = deg[perm]
    dsort = deg_pos.reshape(CORES, NL)
    Lj = np.maximum(dsort[:, ::128].max(axis=0), 1).astype(np.int64)  # [NB]
    offs = np.concatenate([[0], np.cumsum(Lj)]).astype(np.int64)      # [NB+1]
    C_total = int(offs[-1])
    S = 128 * C_total

    # slot for each edge: sorted by destination position
    tpos = pos_of[tgt]
    spos = pos_of[src]
    order = np.argsort(tpos, kind="stable")
    st = tpos[order]
    ss = spos[order]
    starts = np.concatenate([[0], np.cumsum(deg_pos)])
    l = np.arange(E, dtype=np.int64) - starts[st]
    c = st // NL
    ploc = st % NL
    j = ploc // 128
    lane = ploc % 128
    slot = (offs[j] + l) * 128 + lane

    idx_all = np.full((CORES, S), PAD_ROW, dtype=np.int16)
    idx_all[c, slot] = ss.astype(np.int16)

    idx_w = np.empty((CORES, 128, S // 16), dtype=np.int16)
    idx_s = np.empty((CORES, 128, C_total), dtype=np.int16)
    for cc in range(CORES):
        w = idx_all[cc].reshape(S // 16, 16).T  # [16, S//16]
        idx_w[cc] = np.tile(w, (8, 1))
        idx_s[cc] = idx_all[cc].reshape(C_total, 128).T

    # gather chunks in chunk-col space, <=64 cols (8192 idxs) per dma_gather;
    # prefer block-boundary cuts, allow mid-block cuts for oversized blocks
    CAP = 64
    bounds = [0]
    while bounds[-1] < C_total:
        pos = bounds[-1]
        end = min(pos + CAP, C_total)
        cand = [int(o) for o in offs if pos < o <= end]
        bounds.append(max(cand) if cand else end)
    chunks = list(zip(bounds[:-1], bounds[1:]))
    segs = []
    for j in range(NB):
        s = []
        for ci, (c0, c1) in enumerate(chunks):
            lo, hi = max(c0, int(offs[j])), min(c1, int(offs[j + 1]))
            if lo < hi:
                s.append((ci, lo - c0, hi - lo))
        segs.append(s)

    return perm, Lj, offs, C_total, idx_w, idx_s, chunks, segs


def kernel(edge_index, user_emb, item_emb, attn_w, attn_b, causal_adj,
           l0_lw, l0_lb, l0_rw, l1_lw, l1_lb, l1_rw):
    global LAST_RESULTS
    edge_index = np.asarray(edge_index)
    user_emb = np.asarray(user_emb, dtype=np.float32)
    item_emb = np.asarray(item_emb, dtype=np.float32)
    attn_w = np.asarray(attn_w, dtype=np.float32)
    attn_b = np.asarray(attn_b, dtype=np.float32)
    causal_adj = np.asarray(causal_adj, dtype=np.float32)

    perm, Lj, offs, C_total, idx_w, idx_s, chunks, segs = _preprocess(edge_index)

    x0 = np.concatenate([user_emb, item_emb], axis=0)  # [N, H]
    x0p = x0[perm]
    x0aug = np.ascontiguousarray(
        np.concatenate([x0p, np.ones((N, 1), np.float32)], axis=1)
    )
    adjp = np.ascontiguousarray(causal_adj[:, perm])

    wpack = np.zeros((H, 323), dtype=np.float32)
    wpack[:, 0:64] = attn_w.T
    wpack[:, 64:128] = np.asarray(l0_lw, np.float32).T
    wpack[:, 128:192] = np.asarray(l0_rw, np.float32).T
    wpack[:, 192:256] = np.asarray(l1_lw, np.float32).T
    wpack[:, 256:320] = np.asarray(l1_rw, np.float32).T
    wpack[:, 320] = attn_b
    wpack[:, 321] = np.asarray(l0_lb, np.float32)
    wpack[:, 322] = np.asarray(l1_lb, np.float32)

    key = (C_total, tuple(int(x) for x in Lj), tuple(chunks))
    if key not in _NC_CACHE:
        _NC_CACHE[key] = _build_nc(
            C_total, [int(x) for x in Lj], [int(x) for x in offs], chunks, segs
        )
    nc = _NC_CACHE[key]

    in_maps = []
    for c in range(CORES):
        xT0c = np.ascontiguousarray(x0p[c * NL : (c + 1) * NL].T)
        in_maps.append(
            {
                "x0aug": x0aug,
                "xT0": xT0c,
                "adj": adjp,
                "wpack": wpack,
                "idxw": idx_w[c],
                "idxs": idx_s[c],
            }
        )

    import os

    res = run_bass_kernel_spmd(
        nc,
        in_maps,
        core_ids=list(range(CORES)),
        trace=bool(os.environ.get("KERNEL_TRACE")),
    )
    LAST_RESULTS = res

    out_pos = np.concatenate([res.results[c]["out"] for c in range(CORES)], axis=0)
    out = np.empty_like(out_pos)
    out[perm] = out_pos
    return out[:U], out[U:]
